# revision 21
# baseline (speedup 1.0000x reference)
"""MoE layer (8 experts, top-2) for 8 Trainium2 NeuronCores.

Strategy: expert-parallel. Host computes the gate (softmax + top-2) in
numpy — this *is* the sharding decision — then gathers each expert's
tokens into a fixed-capacity batch. Core e runs expert e's FFN
    y = (silu(x @ w1.T) * (x @ w3.T)) @ w2.T
on its gathered tokens in bf16 (fp32 PSUM accumulation). Host combines
with the top-2 probabilities (scatter-add).

Device layout is feature-major ("transposed") throughout so no on-device
transposes are needed:
  xt   [D, C]             bf16  tokens for this expert, d-major
  w1c  [KF, 128, KD, 128] bf16  w1.T chunked by output f-tile
  w3c  [KF, 128, KD, 128] bf16  w3.T chunked by output f-tile
  w2c  [KD, 128, KF, 128] bf16  w2.T chunked by output d-tile
  yt   [D, C]             f32   expert output, d-major
Weight chunking makes each output-tile's weights one contiguous DMA
(2 KB/partition), so compute starts after ~2 MB instead of 13 MB.
"""

import os
import sys
from contextlib import ExitStack

import numpy as np

sys.path.insert(0, "/opt/trn_rl_repo")

import ml_dtypes


def _ensure_axon_hooks():
    """bass_utils imports antenv.axon_hooks when tracing is requested (even
    via a stray BASS_TRACE env var); this container's antenv lacks that
    submodule. Provide a no-op fallback so the import never crashes."""
    import types

    if "antenv.axon_hooks" in sys.modules:
        return
    try:
        from antenv import axon_hooks  # noqa: F401

        return
    except ImportError:
        pass
    mod = types.ModuleType("antenv.axon_hooks")
    _state = {"hook": None}
    mod.get_axon_ntff_profile_hook = lambda: _state["hook"]
    mod.set_axon_ntff_profile_hook = lambda h: _state.__setitem__("hook", h)
    sys.modules["antenv.axon_hooks"] = mod
    try:
        import antenv

        antenv.axon_hooks = mod
    except ImportError:
        pass


_ensure_axon_hooks()

# ---- problem constants (hardcoded; kernel.py must be self-contained) ----
B, T, D, F, E, TOP_K = 8, 2048, 1024, 2048, 8, 2
N = B * T
NCORES = 8
KD = D // 128   # 8  contraction chunks over D
KF = F // 128   # 16 contraction chunks over F

_compiled = {}


def _tok_tiles(C):
    """Token-tile widths: the small remainder first (cheap startup while
    DMAs stream, warms the PE activity monitor), then 512s."""
    sizes = [512] * (C // 512)
    if C % 512:
        sizes.insert(0, C % 512)
    return sizes


def _capacity(maxc):
    """Smallest capacity >= maxc: full 512 tiles + a 16-aligned remainder."""
    full = maxc // 512
    rem = maxc - full * 512
    return full * 512 + -(-rem // 16) * 16


def _build_bass(C: int, act: str = "Silu"):
    """Build the SPMD Bass program for capacity C (tokens per expert)."""
    import concourse.bacc as bacc
    import concourse.tile as tile
    from concourse import mybir

    act_fn = getattr(mybir.ActivationFunctionType, act)

    bf16 = mybir.dt.bfloat16
    f32 = mybir.dt.float32

    nc = bacc.Bacc(
        "TRN2", target_bir_lowering=False, debug=False, num_devices=NCORES
    )
    xt = nc.declare_dram_parameter("xt", [D, C], bf16, isOutput=False)
    w1c = nc.declare_dram_parameter("w1c", [KF, 128, KD, 128], bf16, isOutput=False)
    w3c = nc.declare_dram_parameter("w3c", [KF, 128, KD, 128], bf16, isOutput=False)
    w2c = nc.declare_dram_parameter("w2c", [KD, 128, KF, 128], bf16, isOutput=False)
    yt = nc.declare_dram_parameter("yt", [D, C], f32, isOutput=True)

    xt_r = xt.rearrange("(k p) n -> p k n", p=128)   # [128, KD, C]
    yt_r = yt.rearrange("(k p) n -> p k n", p=128)   # [128, KD, C]

    sizes = _tok_tiles(C)
    starts = np.cumsum([0] + sizes[:-1]).tolist()

    with ExitStack() as ctx:
        tc = ctx.enter_context(tile.TileContext(nc))
        wpool = ctx.enter_context(tc.tile_pool(name="w", bufs=1))
        xpool = ctx.enter_context(tc.tile_pool(name="x", bufs=3))
        hpool = ctx.enter_context(tc.tile_pool(name="h", bufs=2))
        spool = ctx.enter_context(tc.tile_pool(name="s", bufs=3))
        opool = ctx.enter_context(tc.tile_pool(name="o", bufs=4))
        psh = ctx.enter_context(tc.tile_pool(name="psh", bufs=2, space="PSUM"))
        psy = ctx.enter_context(tc.tile_pool(name="psy", bufs=2, space="PSUM"))

        # Weights resident in SBUF for the whole kernel, one tile per
        # output chunk. DMA order: first f-chunk + first x tile first so
        # PE starts ~5µs in; the rest streams behind it.
        w1f = [
            wpool.tile([128, KD, 128], bf16, tag=f"w1f{f}", name=f"w1f{f}")
            for f in range(KF)
        ]
        w3f = [
            wpool.tile([128, KD, 128], bf16, tag=f"w3f{f}", name=f"w3f{f}")
            for f in range(KF)
        ]
        w2d = [
            wpool.tile([128, KF, 128], bf16, tag=f"w2d{d}", name=f"w2d{d}")
            for d in range(KD)
        ]

        def load_x(ts, tw):
            xs = [
                xpool.tile([128, tw], bf16, tag=f"xs{k}", name=f"xs{k}")
                for k in range(KD)
            ]
            for k in range(KD):
                nc.sync.dma_start(xs[k][:], xt_r[:, k, ts])
            return xs

        xs0 = load_x(slice(0, sizes[0]), sizes[0])
        nc.sync.dma_start(w1f[0][:], w1c[0])
        nc.sync.dma_start(w3f[0][:], w3c[0])
        for f in range(1, KF):
            nc.sync.dma_start(w1f[f][:], w1c[f])
            nc.sync.dma_start(w3f[f][:], w3c[f])
        for d in range(KD):
            nc.sync.dma_start(w2d[d][:], w2c[d])

        for t, (t0, tw) in enumerate(zip(starts, sizes)):
            ts = slice(t0, t0 + tw)
            xs = xs0 if t == 0 else load_x(ts, tw)

            hs = [
                hpool.tile([128, tw], bf16, tag=f"hs{f}", name=f"hs{f}")
                for f in range(KF)
            ]
            for f in range(KF):
                ph1 = psh.tile([128, tw], f32, tag="ph1")
                ph3 = psh.tile([128, tw], f32, tag="ph3")
                for k in range(KD):
                    nc.tensor.matmul(
                        ph1[:], w1f[f][:, k, :], xs[k][:],
                        start=(k == 0), stop=(k == KD - 1),
                    )
                for k in range(KD):
                    nc.tensor.matmul(
                        ph3[:], w3f[f][:, k, :], xs[k][:],
                        start=(k == 0), stop=(k == KD - 1),
                    )
                sil = spool.tile([128, tw], f32, tag="sil")
                nc.scalar.activation(sil[:], ph1[:], act_fn)
                nc.vector.tensor_mul(hs[f][:], sil[:], ph3[:])

            for d in range(KD):
                py = psy.tile([128, tw], f32, tag="py")
                for f in range(KF):
                    nc.tensor.matmul(
                        py[:], w2d[d][:, f, :], hs[f][:],
                        start=(f == 0), stop=(f == KF - 1),
                    )
                yo = opool.tile([128, tw], f32, tag="yo")
                nc.vector.tensor_copy(yo[:], py[:])
                nc.sync.dma_start(yt_r[:, d, ts], yo[:])

    nc.compile()
    return nc


def _route(xf: np.ndarray, gate_w: np.ndarray):
    """Numpy replica of the reference gate: softmax + top-2 + renorm."""
    logits = xf @ gate_w.T  # [N, E] f32
    m = logits.max(axis=-1, keepdims=True)
    p = np.exp(logits - m, dtype=np.float32)
    p /= p.sum(axis=-1, keepdims=True)
    i1 = np.argmax(p, axis=-1)
    ar = np.arange(N)
    pm = p.copy()
    pm[ar, i1] = -1.0
    i2 = np.argmax(pm, axis=-1)
    p1 = p[ar, i1]
    p2 = p[ar, i2]
    s = p1 + p2
    return i1, i2, (p1 / s).astype(np.float32), (p2 / s).astype(np.float32)


def _chunk_w13(wt):
    """[D, F] -> [KF, 128, KD, 128]: out[f, p, k, j] = wt[k*128+p, f*128+j]."""
    return np.ascontiguousarray(
        wt.reshape(KD, 128, KF, 128).transpose(2, 1, 0, 3)
    )


def _chunk_w2(wt):
    """[F, D] -> [KD, 128, KF, 128]: out[d, p, k, j] = wt[k*128+p, d*128+j]."""
    return np.ascontiguousarray(
        wt.reshape(KF, 128, KD, 128).transpose(2, 1, 0, 3)
    )


last_results = None  # BassKernelResults of the most recent run (for test harness)


def kernel(x, gate_w, w1, w2, w3):
    from concourse.bass_utils import run_bass_kernel_spmd

    xf = np.ascontiguousarray(np.asarray(x, dtype=np.float32).reshape(N, D))
    gate_w = np.asarray(gate_w, dtype=np.float32)
    i1, i2, c1, c2 = _route(xf, gate_w)

    # per-expert token lists (a token appears at most once per expert)
    idxs, combs = [], []
    for e in range(E):
        a = np.where(i1 == e)[0]
        b = np.where(i2 == e)[0]
        idxs.append(np.concatenate([a, b]))
        combs.append(np.concatenate([c1[a], c2[b]]))
    maxc = max(len(ix) for ix in idxs)
    C = _capacity(maxc)

    if C not in _compiled:
        _compiled[C] = _build_bass(C)
    nc = _compiled[C]

    bf = ml_dtypes.bfloat16
    w1b = np.asarray(w1, dtype=np.float32)
    w2b = np.asarray(w2, dtype=np.float32)
    w3b = np.asarray(w3, dtype=np.float32)

    in_maps = []
    for e in range(E):
        ix = idxs[e]
        xg = np.zeros((C, D), dtype=bf)
        xg[: len(ix)] = xf[ix].astype(bf)
        in_maps.append({
            "xt": np.ascontiguousarray(xg.T),
            "w1c": _chunk_w13(w1b[e].T.astype(bf)),
            "w3c": _chunk_w13(w3b[e].T.astype(bf)),
            "w2c": _chunk_w2(w2b[e].T.astype(bf)),
        })

    trace = os.environ.get("BASS_KERNEL_TRACE", "") not in ("", "0")
    res = run_bass_kernel_spmd(
        nc, in_maps, core_ids=list(range(NCORES)), trace=trace
    )
    global last_results
    last_results = res

    out = np.zeros((N, D), dtype=np.float32)
    for e in range(E):
        ix = idxs[e]
        yT = res.results[e]["yt"]  # [D, C] f32
        out[ix] += combs[e][:, None] * yT.T[: len(ix)]
    return out.reshape(B, T, D)


# revision 23
# speedup vs baseline: 1.0109x; 1.0109x over previous
"""MoE layer (8 experts, top-2) for 8 Trainium2 NeuronCores.

Strategy: expert-parallel. Host computes the gate (softmax + top-2) in
numpy — this *is* the sharding decision — then gathers each expert's
tokens into a fixed-capacity batch. Core e runs expert e's FFN
    y = (silu(x @ w1.T) * (x @ w3.T)) @ w2.T
on its gathered tokens in bf16 (fp32 PSUM accumulation). Host combines
with the top-2 probabilities (scatter-add).

Device layout is feature-major ("transposed") throughout so no on-device
transposes are needed:
  xt   [D, C]             bf16  tokens for this expert, d-major
  w1c  [KF, 128, KD, 128] bf16  w1.T chunked by output f-tile
  w3c  [KF, 128, KD, 128] bf16  w3.T chunked by output f-tile
  w2c  [KD, 128, KF, 128] bf16  w2.T chunked by output d-tile
  yt   [D, C]             f32   expert output, d-major
Weight chunking makes each output-tile's weights one contiguous DMA
(2 KB/partition), so compute starts after ~2 MB instead of 13 MB.
"""

import os
import sys
from contextlib import ExitStack

import numpy as np

sys.path.insert(0, "/opt/trn_rl_repo")

import ml_dtypes


def _ensure_axon_hooks():
    """bass_utils imports antenv.axon_hooks when tracing is requested (even
    via a stray BASS_TRACE env var); this container's antenv lacks that
    submodule. Provide a no-op fallback so the import never crashes."""
    import types

    if "antenv.axon_hooks" in sys.modules:
        return
    try:
        from antenv import axon_hooks  # noqa: F401

        return
    except ImportError:
        pass
    mod = types.ModuleType("antenv.axon_hooks")
    _state = {"hook": None}
    mod.get_axon_ntff_profile_hook = lambda: _state["hook"]
    mod.set_axon_ntff_profile_hook = lambda h: _state.__setitem__("hook", h)
    sys.modules["antenv.axon_hooks"] = mod
    try:
        import antenv

        antenv.axon_hooks = mod
    except ImportError:
        pass


_ensure_axon_hooks()

# ---- problem constants (hardcoded; kernel.py must be self-contained) ----
B, T, D, F, E, TOP_K = 8, 2048, 1024, 2048, 8, 2
N = B * T
NCORES = 8
KD = D // 128   # 8  contraction chunks over D
KF = F // 128   # 16 contraction chunks over F

_compiled = {}


def _tok_tiles(C):
    """Token-tile widths: 512s then one remainder (multiple of 16)."""
    sizes = [512] * (C // 512)
    if C % 512:
        sizes.append(C % 512)
    return sizes


def _capacity(maxc):
    """Smallest capacity >= maxc: full 512 tiles + a 16-aligned remainder."""
    full = maxc // 512
    rem = maxc - full * 512
    return full * 512 + -(-rem // 16) * 16


def _build_bass(C: int, act: str = "Silu"):
    """Build the SPMD Bass program for capacity C (tokens per expert)."""
    import concourse.bacc as bacc
    import concourse.tile as tile
    from concourse import mybir

    act_fn = getattr(mybir.ActivationFunctionType, act)

    bf16 = mybir.dt.bfloat16
    f32 = mybir.dt.float32

    nc = bacc.Bacc(
        "TRN2", target_bir_lowering=False, debug=False, num_devices=NCORES
    )
    xt = nc.declare_dram_parameter("xt", [D, C], bf16, isOutput=False)
    w1c = nc.declare_dram_parameter("w1c", [KF, 128, KD, 128], bf16, isOutput=False)
    w3c = nc.declare_dram_parameter("w3c", [KF, 128, KD, 128], bf16, isOutput=False)
    w2c = nc.declare_dram_parameter("w2c", [KD, 128, KF, 128], bf16, isOutput=False)
    yt = nc.declare_dram_parameter("yt", [D, C], f32, isOutput=True)

    xt_r = xt.rearrange("(k p) n -> p k n", p=128)   # [128, KD, C]
    yt_r = yt.rearrange("(k p) n -> p k n", p=128)   # [128, KD, C]

    sizes = _tok_tiles(C)
    starts = np.cumsum([0] + sizes[:-1]).tolist()

    with ExitStack() as ctx:
        tc = ctx.enter_context(tile.TileContext(nc))
        wpool = ctx.enter_context(tc.tile_pool(name="w", bufs=1))
        xpool = ctx.enter_context(tc.tile_pool(name="x", bufs=3))
        hpool = ctx.enter_context(tc.tile_pool(name="h", bufs=2))
        spool = ctx.enter_context(tc.tile_pool(name="s", bufs=3))
        opool = ctx.enter_context(tc.tile_pool(name="o", bufs=4))
        psh = ctx.enter_context(tc.tile_pool(name="psh", bufs=2, space="PSUM"))
        psy = ctx.enter_context(tc.tile_pool(name="psy", bufs=2, space="PSUM"))

        # Weights resident in SBUF for the whole kernel, one tile per
        # output chunk. DMA order: first f-chunk + first x tile first so
        # PE starts ~5µs in; the rest streams behind it.
        w1f = [
            wpool.tile([128, KD, 128], bf16, tag=f"w1f{f}", name=f"w1f{f}")
            for f in range(KF)
        ]
        w3f = [
            wpool.tile([128, KD, 128], bf16, tag=f"w3f{f}", name=f"w3f{f}")
            for f in range(KF)
        ]
        w2d = [
            wpool.tile([128, KF, 128], bf16, tag=f"w2d{d}", name=f"w2d{d}")
            for d in range(KD)
        ]

        def load_x(ts, tw):
            xs = [
                xpool.tile([128, tw], bf16, tag=f"xs{k}", name=f"xs{k}")
                for k in range(KD)
            ]
            for k in range(KD):
                nc.sync.dma_start(xs[k][:], xt_r[:, k, ts])
            return xs

        xs0 = load_x(slice(0, sizes[0]), sizes[0])
        nc.sync.dma_start(w1f[0][:], w1c[0])
        nc.sync.dma_start(w3f[0][:], w3c[0])
        for f in range(1, KF):
            nc.sync.dma_start(w1f[f][:], w1c[f])
            nc.sync.dma_start(w3f[f][:], w3c[f])
        for d in range(KD):
            nc.sync.dma_start(w2d[d][:], w2c[d])

        for t, (t0, tw) in enumerate(zip(starts, sizes)):
            ts = slice(t0, t0 + tw)
            xs = xs0 if t == 0 else load_x(ts, tw)

            hs = hpool.tile([128, KF, tw], bf16, tag="hs")
            for f in range(KF):
                ph1 = psh.tile([128, tw], f32, tag="ph1")
                ph3 = psh.tile([128, tw], f32, tag="ph3")
                for k in range(KD):
                    nc.tensor.matmul(
                        ph1[:], w1f[f][:, k, :], xs[k][:],
                        start=(k == 0), stop=(k == KD - 1),
                    )
                for k in range(KD):
                    nc.tensor.matmul(
                        ph3[:], w3f[f][:, k, :], xs[k][:],
                        start=(k == 0), stop=(k == KD - 1),
                    )
                sil = spool.tile([128, tw], f32, tag="sil")
                nc.scalar.activation(sil[:], ph1[:], act_fn)
                nc.vector.tensor_mul(hs[:, f, :], sil[:], ph3[:])

            for d in range(KD):
                py = psy.tile([128, tw], f32, tag="py")
                for f in range(KF):
                    nc.tensor.matmul(
                        py[:], w2d[d][:, f, :], hs[:, f, :],
                        start=(f == 0), stop=(f == KF - 1),
                    )
                yo = opool.tile([128, tw], f32, tag="yo")
                nc.vector.tensor_copy(yo[:], py[:])
                nc.sync.dma_start(yt_r[:, d, ts], yo[:])

    nc.compile()
    return nc


def _route(xf: np.ndarray, gate_w: np.ndarray):
    """Numpy replica of the reference gate: softmax + top-2 + renorm."""
    logits = xf @ gate_w.T  # [N, E] f32
    m = logits.max(axis=-1, keepdims=True)
    p = np.exp(logits - m, dtype=np.float32)
    p /= p.sum(axis=-1, keepdims=True)
    i1 = np.argmax(p, axis=-1)
    ar = np.arange(N)
    pm = p.copy()
    pm[ar, i1] = -1.0
    i2 = np.argmax(pm, axis=-1)
    p1 = p[ar, i1]
    p2 = p[ar, i2]
    s = p1 + p2
    return i1, i2, (p1 / s).astype(np.float32), (p2 / s).astype(np.float32)


def _chunk_w13(wt):
    """[D, F] -> [KF, 128, KD, 128]: out[f, p, k, j] = wt[k*128+p, f*128+j]."""
    return np.ascontiguousarray(
        wt.reshape(KD, 128, KF, 128).transpose(2, 1, 0, 3)
    )


def _chunk_w2(wt):
    """[F, D] -> [KD, 128, KF, 128]: out[d, p, k, j] = wt[k*128+p, d*128+j]."""
    return np.ascontiguousarray(
        wt.reshape(KF, 128, KD, 128).transpose(2, 1, 0, 3)
    )


last_results = None  # BassKernelResults of the most recent run (for test harness)


def kernel(x, gate_w, w1, w2, w3):
    from concourse.bass_utils import run_bass_kernel_spmd

    xf = np.ascontiguousarray(np.asarray(x, dtype=np.float32).reshape(N, D))
    gate_w = np.asarray(gate_w, dtype=np.float32)
    i1, i2, c1, c2 = _route(xf, gate_w)

    # per-expert token lists (a token appears at most once per expert)
    idxs, combs = [], []
    for e in range(E):
        a = np.where(i1 == e)[0]
        b = np.where(i2 == e)[0]
        idxs.append(np.concatenate([a, b]))
        combs.append(np.concatenate([c1[a], c2[b]]))
    maxc = max(len(ix) for ix in idxs)
    C = _capacity(maxc)

    if C not in _compiled:
        _compiled[C] = _build_bass(C)
    nc = _compiled[C]

    bf = ml_dtypes.bfloat16
    w1b = np.asarray(w1, dtype=np.float32)
    w2b = np.asarray(w2, dtype=np.float32)
    w3b = np.asarray(w3, dtype=np.float32)

    in_maps = []
    for e in range(E):
        ix = idxs[e]
        xg = np.zeros((C, D), dtype=bf)
        xg[: len(ix)] = xf[ix].astype(bf)
        in_maps.append({
            "xt": np.ascontiguousarray(xg.T),
            "w1c": _chunk_w13(w1b[e].T.astype(bf)),
            "w3c": _chunk_w13(w3b[e].T.astype(bf)),
            "w2c": _chunk_w2(w2b[e].T.astype(bf)),
        })

    trace = os.environ.get("BASS_KERNEL_TRACE", "") not in ("", "0")
    res = run_bass_kernel_spmd(
        nc, in_maps, core_ids=list(range(NCORES)), trace=trace
    )
    global last_results
    last_results = res

    out = np.zeros((N, D), dtype=np.float32)
    for e in range(E):
        ix = idxs[e]
        yT = res.results[e]["yt"]  # [D, C] f32
        out[ix] += combs[e][:, None] * yT.T[: len(ix)]
    return out.reshape(B, T, D)


# revision 24
# speedup vs baseline: 1.0147x; 1.0037x over previous
"""MoE layer (8 experts, top-2) for 8 Trainium2 NeuronCores.

Strategy: expert-parallel. Host computes the gate (softmax + top-2) in
numpy — this *is* the sharding decision — then gathers each expert's
tokens into a fixed-capacity batch. Core e runs expert e's FFN
    y = (silu(x @ w1.T) * (x @ w3.T)) @ w2.T
on its gathered tokens in bf16 (fp32 PSUM accumulation). Host combines
with the top-2 probabilities (scatter-add).

Device layout is feature-major ("transposed") throughout so no on-device
transposes are needed:
  xt   [D, C]             bf16  tokens for this expert, d-major
  w1c  [KF, 128, KD, 128] bf16  w1.T chunked by output f-tile
  w3c  [KF, 128, KD, 128] bf16  w3.T chunked by output f-tile
  w2c  [KD, 128, KF, 128] bf16  w2.T chunked by output d-tile
  yt   [D, C]             f32   expert output, d-major
Weight chunking makes each output-tile's weights one contiguous DMA
(2 KB/partition), so compute starts after ~2 MB instead of 13 MB.
"""

import os
import sys
from contextlib import ExitStack

import numpy as np

sys.path.insert(0, "/opt/trn_rl_repo")

import ml_dtypes


def _ensure_axon_hooks():
    """bass_utils imports antenv.axon_hooks when tracing is requested (even
    via a stray BASS_TRACE env var); this container's antenv lacks that
    submodule. Provide a no-op fallback so the import never crashes."""
    import types

    if "antenv.axon_hooks" in sys.modules:
        return
    try:
        from antenv import axon_hooks  # noqa: F401

        return
    except ImportError:
        pass
    mod = types.ModuleType("antenv.axon_hooks")
    _state = {"hook": None}
    mod.get_axon_ntff_profile_hook = lambda: _state["hook"]
    mod.set_axon_ntff_profile_hook = lambda h: _state.__setitem__("hook", h)
    sys.modules["antenv.axon_hooks"] = mod
    try:
        import antenv

        antenv.axon_hooks = mod
    except ImportError:
        pass


_ensure_axon_hooks()

# ---- problem constants (hardcoded; kernel.py must be self-contained) ----
B, T, D, F, E, TOP_K = 8, 2048, 1024, 2048, 8, 2
N = B * T
NCORES = 8
KD = D // 128   # 8  contraction chunks over D
KF = F // 128   # 16 contraction chunks over F

_compiled = {}


def _tok_tiles(C):
    """Token-tile widths: 512s then one remainder (multiple of 16)."""
    sizes = [512] * (C // 512)
    if C % 512:
        sizes.append(C % 512)
    return sizes


def _capacity(maxc):
    """Smallest capacity >= maxc: full 512 tiles + a 16-aligned remainder."""
    full = maxc // 512
    rem = maxc - full * 512
    return full * 512 + -(-rem // 16) * 16


def _build_bass(C: int, act: str = "Silu"):
    """Build the SPMD Bass program for capacity C (tokens per expert)."""
    import concourse.bacc as bacc
    import concourse.tile as tile
    from concourse import mybir

    act_fn = getattr(mybir.ActivationFunctionType, act)

    bf16 = mybir.dt.bfloat16
    f32 = mybir.dt.float32

    nc = bacc.Bacc(
        "TRN2", target_bir_lowering=False, debug=False, num_devices=NCORES
    )
    xt = nc.declare_dram_parameter("xt", [D, C], bf16, isOutput=False)
    w1c = nc.declare_dram_parameter("w1c", [KF, 128, KD, 128], bf16, isOutput=False)
    w3c = nc.declare_dram_parameter("w3c", [KF, 128, KD, 128], bf16, isOutput=False)
    w2c = nc.declare_dram_parameter("w2c", [KD, 128, KF, 128], bf16, isOutput=False)
    yt = nc.declare_dram_parameter("yt", [D, C], f32, isOutput=True)

    xt_r = xt.rearrange("(k p) n -> p k n", p=128)   # [128, KD, C]
    yt_r = yt.rearrange("(k p) n -> p k n", p=128)   # [128, KD, C]

    sizes = _tok_tiles(C)
    starts = np.cumsum([0] + sizes[:-1]).tolist()

    with ExitStack() as ctx:
        tc = ctx.enter_context(tile.TileContext(nc))
        wpool = ctx.enter_context(tc.tile_pool(name="w", bufs=1))
        xpool = ctx.enter_context(tc.tile_pool(name="x", bufs=3))
        hpool = ctx.enter_context(tc.tile_pool(name="h", bufs=2))
        spool = ctx.enter_context(tc.tile_pool(name="s", bufs=3))
        opool = ctx.enter_context(tc.tile_pool(name="o", bufs=4))
        psh = ctx.enter_context(tc.tile_pool(name="psh", bufs=2, space="PSUM"))
        psy = ctx.enter_context(tc.tile_pool(name="psy", bufs=2, space="PSUM"))
        warmp = ctx.enter_context(tc.tile_pool(name="warmp", bufs=1))
        warmps = ctx.enter_context(tc.tile_pool(name="warmps", bufs=1, space="PSUM"))

        # PE warm-up: the first ~15µs are DMA-bound, which lets the HAM
        # clock gate go cold (1.2 GHz) for the first ~3.4µs of real
        # matmuls. Keep the PE busy on a scratch tile meanwhile; output
        # is never read and gates nothing.
        wsrc = warmp.tile([128, 512], bf16)
        nc.gpsimd.memset(wsrc[:], 0.0)
        wdst = warmps.tile([128, 512], f32)
        for _ in range(34):
            nc.tensor.matmul(wdst[:], wsrc[:, 0:128], wsrc[:], start=True, stop=True)

        # Weights resident in SBUF for the whole kernel, one tile per
        # output chunk. DMA order: first f-chunk + first x tile first so
        # PE starts ~5µs in; the rest streams behind it.
        w1f = [
            wpool.tile([128, KD, 128], bf16, tag=f"w1f{f}", name=f"w1f{f}")
            for f in range(KF)
        ]
        w3f = [
            wpool.tile([128, KD, 128], bf16, tag=f"w3f{f}", name=f"w3f{f}")
            for f in range(KF)
        ]
        w2d = [
            wpool.tile([128, KF, 128], bf16, tag=f"w2d{d}", name=f"w2d{d}")
            for d in range(KD)
        ]

        def load_x(ts, tw):
            xs = [
                xpool.tile([128, tw], bf16, tag=f"xs{k}", name=f"xs{k}")
                for k in range(KD)
            ]
            for k in range(KD):
                nc.sync.dma_start(xs[k][:], xt_r[:, k, ts])
            return xs

        xs0 = load_x(slice(0, sizes[0]), sizes[0])
        nc.sync.dma_start(w1f[0][:], w1c[0])
        nc.sync.dma_start(w3f[0][:], w3c[0])
        for f in range(1, KF):
            nc.sync.dma_start(w1f[f][:], w1c[f])
            nc.sync.dma_start(w3f[f][:], w3c[f])
        for d in range(KD):
            nc.sync.dma_start(w2d[d][:], w2c[d])

        for t, (t0, tw) in enumerate(zip(starts, sizes)):
            ts = slice(t0, t0 + tw)
            xs = xs0 if t == 0 else load_x(ts, tw)

            hs = hpool.tile([128, KF, tw], bf16, tag="hs")
            for f in range(KF):
                ph1 = psh.tile([128, tw], f32, tag="ph1")
                ph3 = psh.tile([128, tw], f32, tag="ph3")
                for k in range(KD):
                    nc.tensor.matmul(
                        ph1[:], w1f[f][:, k, :], xs[k][:],
                        start=(k == 0), stop=(k == KD - 1),
                    )
                for k in range(KD):
                    nc.tensor.matmul(
                        ph3[:], w3f[f][:, k, :], xs[k][:],
                        start=(k == 0), stop=(k == KD - 1),
                    )
                sil = spool.tile([128, tw], f32, tag="sil")
                nc.scalar.activation(sil[:], ph1[:], act_fn)
                nc.vector.tensor_mul(hs[:, f, :], sil[:], ph3[:])

            for d in range(KD):
                py = psy.tile([128, tw], f32, tag="py")
                for f in range(KF):
                    nc.tensor.matmul(
                        py[:], w2d[d][:, f, :], hs[:, f, :],
                        start=(f == 0), stop=(f == KF - 1),
                    )
                yo = opool.tile([128, tw], f32, tag="yo")
                nc.vector.tensor_copy(yo[:], py[:])
                nc.sync.dma_start(yt_r[:, d, ts], yo[:])

    nc.compile()
    return nc


def _route(xf: np.ndarray, gate_w: np.ndarray):
    """Numpy replica of the reference gate: softmax + top-2 + renorm."""
    logits = xf @ gate_w.T  # [N, E] f32
    m = logits.max(axis=-1, keepdims=True)
    p = np.exp(logits - m, dtype=np.float32)
    p /= p.sum(axis=-1, keepdims=True)
    i1 = np.argmax(p, axis=-1)
    ar = np.arange(N)
    pm = p.copy()
    pm[ar, i1] = -1.0
    i2 = np.argmax(pm, axis=-1)
    p1 = p[ar, i1]
    p2 = p[ar, i2]
    s = p1 + p2
    return i1, i2, (p1 / s).astype(np.float32), (p2 / s).astype(np.float32)


def _chunk_w13(wt):
    """[D, F] -> [KF, 128, KD, 128]: out[f, p, k, j] = wt[k*128+p, f*128+j]."""
    return np.ascontiguousarray(
        wt.reshape(KD, 128, KF, 128).transpose(2, 1, 0, 3)
    )


def _chunk_w2(wt):
    """[F, D] -> [KD, 128, KF, 128]: out[d, p, k, j] = wt[k*128+p, d*128+j]."""
    return np.ascontiguousarray(
        wt.reshape(KF, 128, KD, 128).transpose(2, 1, 0, 3)
    )


last_results = None  # BassKernelResults of the most recent run (for test harness)


def kernel(x, gate_w, w1, w2, w3):
    from concourse.bass_utils import run_bass_kernel_spmd

    xf = np.ascontiguousarray(np.asarray(x, dtype=np.float32).reshape(N, D))
    gate_w = np.asarray(gate_w, dtype=np.float32)
    i1, i2, c1, c2 = _route(xf, gate_w)

    # per-expert token lists (a token appears at most once per expert)
    idxs, combs = [], []
    for e in range(E):
        a = np.where(i1 == e)[0]
        b = np.where(i2 == e)[0]
        idxs.append(np.concatenate([a, b]))
        combs.append(np.concatenate([c1[a], c2[b]]))
    maxc = max(len(ix) for ix in idxs)
    C = _capacity(maxc)

    if C not in _compiled:
        _compiled[C] = _build_bass(C)
    nc = _compiled[C]

    bf = ml_dtypes.bfloat16
    w1b = np.asarray(w1, dtype=np.float32)
    w2b = np.asarray(w2, dtype=np.float32)
    w3b = np.asarray(w3, dtype=np.float32)

    in_maps = []
    for e in range(E):
        ix = idxs[e]
        xg = np.zeros((C, D), dtype=bf)
        xg[: len(ix)] = xf[ix].astype(bf)
        in_maps.append({
            "xt": np.ascontiguousarray(xg.T),
            "w1c": _chunk_w13(w1b[e].T.astype(bf)),
            "w3c": _chunk_w13(w3b[e].T.astype(bf)),
            "w2c": _chunk_w2(w2b[e].T.astype(bf)),
        })

    trace = os.environ.get("BASS_KERNEL_TRACE", "") not in ("", "0")
    res = run_bass_kernel_spmd(
        nc, in_maps, core_ids=list(range(NCORES)), trace=trace
    )
    global last_results
    last_results = res

    out = np.zeros((N, D), dtype=np.float32)
    for e in range(E):
        ix = idxs[e]
        yT = res.results[e]["yt"]  # [D, C] f32
        out[ix] += combs[e][:, None] * yT.T[: len(ix)]
    return out.reshape(B, T, D)
